# revision 9
# baseline (speedup 1.0000x reference)
"""ChameleonAttention on 8 Trainium2 NeuronCores.

Tensor-parallel over heads: each core owns 4 of the 32 heads.
  - Wq/Wk/Wv sharded column-wise (512 cols/core), Wo row-wise (512 rows/core)
  - per-head LayerNorm + RoPE computed on-chip, gamma/beta replicated
  - causal attention with block-skipping (only lower-triangular key tiles)
  - per-core partial output [S, HID] summed on host (the TP all-reduce)

v3 engine balance (cost-model driven):
  - PE ~519us is the floor; every other engine is kept under it
  - projection operands fp16 (same PE rate as f32r, half DMA/SBUF)
  - denominator via DVE/fp16 u-sum + one ones-matmul per (head, bank)
  - exp batched over [128,1024] two-bank PSUM score groups
  - O-projection chains interleaved between attention head-blocks,
    offset by one query bank, so PE fills ACT-exp bubbles
  - PSUM->SBUF copies on ACT (scalar.copy), RoPE t24 on gpsimd, rest of
    RoPE + LN on DVE: DVE ~280us, ACT ~250us, Pool ~65us
  - DMAs merged (w per k-panel, rope per half, wo per n-block, out per
    m-tile) to cut SP dispatch serialization from ~290us to ~70us

Softmax uses exp(s*scale - 4) with no running max (LayerNormed q/k bound
the logits), division deferred to after the P@V accumulation.

RoPE is folded with the LayerNorm affine on the host:
  q'[s,j] = xn[s,j]*C1[s,j] + xn[s,p(j)]*D[s,j] + E[s,j]
with C1 = gamma*cos, D = sign*gamma[perm]*sin, E = beta*cos +
sign*beta[perm]*sin.
"""
import math
from contextlib import ExitStack

import numpy as np

_S = 2048
_HID = 4096
_D = 128
_NC = 8
_CPW = _HID // _NC  # columns per core (512) = 4 heads
_HPC = _CPW // _D  # heads per core (4)
_KP = 1024  # contraction panel (8 k-tiles of 128)
_NPAN = _HID // _KP  # 4 panels
_ROPE_THETA = 10000.0
_EPS = 1e-5
_EXP_BIAS = -4.0

_cache = {}


def _build(S, niter=1, phases=('p', 'a', 'o'), knobs=None):
    kb = {'proj': 14, 'wpool': 7, 'pps': 3, 'tps': 4, 'upool': 4, 'sps': 2,
          'ops': 2, 'dxps': 2, 'lnbig': 3, 'lntmp': 4, 'ropep': 1,
          'wop': 8, 'outp': 2}
    kb.update(knobs or {})
    import concourse.tile as tile
    from concourse import bacc, mybir
    from concourse.masks import make_identity

    f32 = mybir.dt.float32
    f16 = mybir.dt.float16
    mul = mybir.AluOpType.mult
    add = mybir.AluOpType.add

    NM = S // 128  # s-tiles (16)
    NQB = S // 512  # query banks (4)
    NMH = NM // 2  # s-tiles per half (8)
    SH = S // 2  # rows per half
    KPT = _KP // 128  # k-tiles per panel (8)
    NN = _HID // 512  # output column blocks (8)

    nc = bacc.Bacc("TRN2", target_bir_lowering=False, debug=False)

    hT_d = nc.dram_tensor("hT", [_HID, S], f16, kind="ExternalInput")
    w_d = {
        t: nc.dram_tensor(f"w{t}", [_HID, _CPW], f16, kind="ExternalInput")
        for t in ("q", "k", "v")
    }
    wo_d = nc.dram_tensor("wo", [_CPW, _HID], f16, kind="ExternalInput")
    rope_d = {}
    for t in ("q", "k"):
        for nm in ("c1", "dd", "ee"):
            rope_d[nm + t] = nc.dram_tensor(
                f"{nm}{t}", [S, _D], f16, kind="ExternalInput"
            )
    masks_d = nc.dram_tensor("masks", [4, 128, 512], f16, kind="ExternalInput")
    out_d = nc.dram_tensor("out", [S, _HID], f16, kind="ExternalOutput")

    half_d = _D // 2

    for _it in range(niter):
      _p = f'i{_it}_' if niter > 1 else ''
      with tile.TileContext(nc) as tc, ExitStack() as ctx:
          # ---- persistent small constants ----
          persist = ctx.enter_context(tc.tile_pool(name=f"{_p}persist", bufs=1))
          ident16 = persist.tile([128, 128], f16)
          make_identity(nc, ident16[:])
          ones16 = persist.tile([128, 128], f16)
          nc.vector.memset(ones16[:], 1.0)
          ebias = persist.tile([128, 1], f32)
          nc.vector.memset(ebias[:], _EXP_BIAS)
          epst = persist.tile([128, 1], f32)
          nc.vector.memset(epst[:], _EPS)

          # ---- fp16 attention operands, filled by phase P ----
          att = ctx.enter_context(tc.tile_pool(name=f"{_p}att", bufs=1))
          qt_a = att.tile([128, _HPC, S], f16, name=f"{_p}qt")
          kt_a = att.tile([128, _HPC, S], f16, name=f"{_p}kt")
          v16 = att.tile([128, NM, 512], f16)

          wop = ctx.enter_context(tc.tile_pool(name=f"{_p}wop", bufs=kb["wop"]))
          wo_t = []  # wo_t[n] = [128, 4, 512]: Wo[k4*128+p, n*512+j]
          mpool = ctx.enter_context(tc.tile_pool(name=f"{_p}mpool", bufs=1))
          mask_t = mpool.tile([128, 4, 512], f16)
          nc.sync.dma_start(mask_t[:], masks_d.ap().rearrange("t p n -> p t n"))

          # ================= phase P: QKV projection =================
          with ExitStack() as pctx:
              acc_pool = pctx.enter_context(tc.tile_pool(name=f"{_p}acc", bufs=1))
              acc = {}
              for t in ("q", "k", "v"):
                  for m in range(NMH):
                      acc[(t, m)] = acc_pool.tile(
                          [128, 512], f16, name=f"{_p}acc_{t}{m}"
                      )
              proj = pctx.enter_context(tc.tile_pool(name=f"{_p}proj", bufs=kb["proj"]))
              wpool = pctx.enter_context(tc.tile_pool(name=f"{_p}wpool", bufs=kb["wpool"]))
              lnbig = pctx.enter_context(tc.tile_pool(name=f"{_p}lnbig", bufs=kb["lnbig"]))
              lntmp = pctx.enter_context(tc.tile_pool(name=f"{_p}lntmp", bufs=kb["lntmp"]))
              ropep = pctx.enter_context(tc.tile_pool(name=f"{_p}ropep", bufs=kb["ropep"]))
              pps = pctx.enter_context(tc.tile_pool(name=f"{_p}pps", bufs=kb["pps"], space="PSUM"))
              tps = pctx.enter_context(tc.tile_pool(name=f"{_p}tps", bufs=kb["tps"], space="PSUM"))

              # rope coeff tiles, one merged DMA per (coeff, tensor, half):
              # [128, 8, 128] with free dim (m, d)
              rope_t = {}

              def _load_rope(half):
                  for t in ("q", "k"):
                      for nm in ("c1", "dd", "ee"):
                          tl = ropep.tile([128, NMH, _D], f16, tag=f"{nm}{t}",
                                          name=f"{_p}rope_{nm}{t}{half}")
                          nc.sync.dma_start(
                              tl[:],
                              rope_d[nm + t][half * SH : (half + 1) * SH, :]
                              .rearrange("(m p) d -> p m d", p=128),
                          )
                          rope_t[(nm, t, half)] = tl

              def _ln_rope(t, half, m):
                  dst = qt_a[:] if t == "q" else kt_a[:]
                  gm = half * NMH + m
                  c1 = rope_t[("c1", t, half)][:, m, :]
                  dd = rope_t[("dd", t, half)][:, m, :]
                  ee = rope_t[("ee", t, half)][:, m, :]
                  xn4 = lnbig.tile([128, _HPC, _D], f16, tag="xn4",
                                   name=f"{_p}xn4_{half}{t}{m}")
                  for h in range(_HPC):
                      x = acc[(t, m)][:, h * _D : (h + 1) * _D]
                      st = lntmp.tile([128, 6], f32, tag="st",
                                      name=f"{_p}st_{half}{t}{m}{h}")
                      mv = lntmp.tile([128, 2], f32, tag="mv",
                                      name=f"{_p}mv_{half}{t}{m}{h}")
                      nc.vector.bn_stats(out=st[:], in_=x)
                      nc.vector.bn_aggr(out=mv[:], in_=st[:])
                      rstd = lntmp.tile([128, 1], f32, tag="rs",
                                        name=f"{_p}rs_{half}{t}{m}{h}")
                      nc.scalar.activation(
                          out=rstd[:], in_=mv[:, 1:2],
                          func=mybir.ActivationFunctionType.Sqrt,
                          bias=epst[:], scale=1.0,
                      )
                      nc.vector.reciprocal(out=rstd[:], in_=rstd[:])
                      nc.vector.tensor_scalar(
                          out=xn4[:, h, :], in0=x,
                          scalar1=mv[:, 0:1], scalar2=rstd[:],
                          op0=mybir.AluOpType.subtract, op1=mul,
                      )
                  # rope: q' = xn*C1 + rot(xn)*D + E, rot via shifted slices.
                  # t24 (gpsimd): rot(xn)*D + E ; q14 (DVE): xn*C1 + t24
                  t24 = lnbig.tile([128, _HPC, _D], f16, tag="t24",
                                   name=f"{_p}t24_{half}{t}{m}")
                  q14 = lnbig.tile([128, _HPC, _D], f16, tag="q14",
                                   name=f"{_p}q14_{half}{t}{m}")
                  c1b = c1.unsqueeze(1).broadcast_to((128, _HPC, _D))
                  ddlo = dd[:, :half_d].unsqueeze(1) \
                      .broadcast_to((128, _HPC, half_d))
                  ddhi = dd[:, half_d:].unsqueeze(1) \
                      .broadcast_to((128, _HPC, half_d))
                  eeb = ee.unsqueeze(1).broadcast_to((128, _HPC, _D))
                  nc.gpsimd.tensor_tensor(
                      t24[:, :, :half_d], xn4[:, :, half_d:], ddlo, op=mul
                  )
                  nc.gpsimd.tensor_tensor(
                      t24[:, :, half_d:], xn4[:, :, :half_d], ddhi, op=mul
                  )
                  nc.gpsimd.tensor_tensor(t24[:], t24[:], eeb, op=add)
                  nc.vector.tensor_tensor(q14[:], xn4[:], c1b, op=mul)
                  nc.vector.tensor_tensor(q14[:], q14[:], t24[:], op=add)
                  tp = tps.tile([128, _HPC, _D], f16, tag="tp",
                                name=f"{_p}tp_{half}{t}{m}")
                  for h in range(_HPC):
                      nc.tensor.transpose(tp[:, h, :], q14[:, h, :], ident16[:])
                  nc.scalar.copy(dst[:, :, gm * 128 : (gm + 1) * 128], tp[:])

              early_w = {}
              for half in range(2):
                  for kp in range(_NPAN):
                      if kp == 1:
                          _load_rope(half)
                      if half == 0 and kp == 0:
                          for wh in range(2):
                              wt = wpool.tile([128, KPT // 2, 512], f16, tag="w",
                                              name=f"{_p}wq_early_{wh}")
                              nc.sync.dma_start(
                                  wt[:],
                                  w_d["q"][wh * (_KP // 2) : (wh + 1) * (_KP // 2), :]
                                  .rearrange("(k p) n -> p k n", p=128),
                              )
                              early_w[wh] = wt
                      hts = []
                      for k4 in range(KPT):
                          ht = proj.tile([128, SH], f16, tag="ht",
                                         name=f"{_p}ht_{half}_{kp}_{k4}")
                          kk = kp * KPT + k4
                          nc.sync.dma_start(
                              ht[:],
                              hT_d[kk * 128 : (kk + 1) * 128,
                                   half * SH : (half + 1) * SH],
                          )
                          hts.append(ht)
                      if half == 0 and kp == 1:
                          # Wo loads: early enough to be resident by phase
                          # A/O, late enough not to delay the first panel
                          for n in range(NN):
                              t_ = wop.tile([128, _HPC, 512], f16, tag="wo",
                                            name=f"{_p}wo_{n}")
                              nc.sync.dma_start(
                                  t_[:],
                                  wo_d[:, n * 512 : (n + 1) * 512]
                                  .rearrange("(k p) n -> p k n", p=128),
                              )
                              wo_t.append(t_)
                      for t in ("q", "k", "v"):
                          # merged weight panels: 2 x [128, 4, 512] = (k4, n)
                          if half == 0 and kp == 0 and t == "q":
                              wts = [early_w[0], early_w[1]]
                          else:
                              wts = []
                              for wh in range(2):
                                  wt = wpool.tile([128, KPT // 2, 512], f16, tag="w",
                                                  name=f"{_p}w{t}_{half}_{kp}_{wh}")
                                  r0 = kp * _KP + wh * (_KP // 2)
                                  nc.sync.dma_start(
                                      wt[:],
                                      w_d[t][r0 : r0 + _KP // 2, :]
                                      .rearrange("(k p) n -> p k n", p=128),
                                  )
                                  wts.append(wt)
                          for m in range(NMH):
                              ps = pps.tile([128, 512], f32, tag="ps",
                                            name=f"{_p}ps_{half}_{kp}_{t}_{m}")
                              for k4 in range(KPT):
                                  nc.tensor.matmul(
                                      ps[:],
                                      hts[k4][:, m * 128 : (m + 1) * 128],
                                      wts[k4 // 4][:, k4 % 4, :],
                                      start=(k4 == 0),
                                      stop=(k4 == KPT - 1),
                                  )
                              last = kp == _NPAN - 1
                              if kp == 0:
                                  nc.scalar.copy(acc[(t, m)][:], ps[:])
                              elif last and t == "v":
                                  # final V panel: fuse add + fp16 cast
                                  gm = half * NMH + m
                                  nc.vector.tensor_tensor(
                                      v16[:, gm, :], acc[(t, m)][:], ps[:], op=add
                                  )
                              else:
                                  nc.vector.tensor_tensor(
                                      acc[(t, m)][:], acc[(t, m)][:], ps[:], op=add
                                  )
                              if last and t != "v":
                                  _ln_rope(t, half, m)

          # ============ phases A+O, O interleaved one bank behind ============
          if 'a' not in phases and 'o' not in phases:
              continue
          with ExitStack() as actx:
              aop = actx.enter_context(tc.tile_pool(name=f"{_p}aop", bufs=1))
              at_a = aop.tile([128, _HPC, S], f16, name=f"{_p}at")
              if 'a' not in phases:
                  nc.vector.memset(at_a[:], 0.0)

              upool = actx.enter_context(tc.tile_pool(name=f"{_p}upool", bufs=kb["upool"]))
              rpool = actx.enter_context(tc.tile_pool(name=f"{_p}rpool", bufs=2))
              outp = actx.enter_context(tc.tile_pool(name=f"{_p}outp", bufs=kb["outp"]))
              sps = actx.enter_context(tc.tile_pool(name=f"{_p}sps", bufs=kb["sps"], space="PSUM"))
              ops = actx.enter_context(tc.tile_pool(name=f"{_p}ops", bufs=kb["ops"], space="PSUM"))
              dxps = actx.enter_context(tc.tile_pool(name=f"{_p}dxps", bufs=kb["dxps"], space="PSUM"))

              scale = 1.0 / math.sqrt(_D)

              def _attn_head(h, qb):
                  nkt = 4 * qb + 4
                  npair = nkt // 2
                  o_ps = ops.tile([128, 512], f32, tag="o", name=f"{_p}o_{h}_{qb}")
                  usum = upool.tile([128, 512], f16, tag="usum",
                                    name=f"{_p}us_{h}_{qb}")
                  for pr in range(npair):
                      sg = sps.tile([128, 1024], f32, tag="s",
                                    name=f"{_p}s_{h}_{qb}_{pr}")
                      for j in range(2):
                          kt = 2 * pr + j
                          nc.tensor.matmul(
                              sg[:, j * 512 : (j + 1) * 512],
                              kt_a[:, h, kt * 128 : (kt + 1) * 128],
                              qt_a[:, h, qb * 512 : (qb + 1) * 512],
                              start=True, stop=True,
                          )
                      u = upool.tile([128, 1024], f16, tag="u",
                                     name=f"{_p}u_{h}_{qb}_{pr}")
                      nc.scalar.activation(
                          out=u[:], in_=sg[:],
                          func=mybir.ActivationFunctionType.Exp,
                          bias=ebias[:], scale=scale,
                      )
                      for j in range(2):
                          kt = 2 * pr + j
                          toff = kt - 4 * qb
                          if toff >= 0:
                              nc.vector.tensor_tensor(
                                  u[:, j * 512 : (j + 1) * 512],
                                  u[:, j * 512 : (j + 1) * 512],
                                  mask_t[:, toff, :], op=mul,
                              )
                      if pr == 0:
                          nc.vector.tensor_tensor(
                              usum[:], u[:, :512], u[:, 512:], op=add
                          )
                      else:
                          for j in range(2):
                              nc.vector.tensor_tensor(
                                  usum[:], usum[:],
                                  u[:, j * 512 : (j + 1) * 512], op=add,
                              )
                      for j in range(2):
                          kt = 2 * pr + j
                          nc.tensor.matmul(
                              o_ps[:],
                              v16[:, kt, h * _D : (h + 1) * _D],
                              u[:, j * 512 : (j + 1) * 512],
                              start=(kt == 0), stop=(kt == nkt - 1),
                          )
                  d_ps = dxps.tile([128, 512], f32, tag="dx",
                                   name=f"{_p}d_{h}_{qb}")
                  nc.tensor.matmul(
                      d_ps[:], ones16[:], usum[:], start=True, stop=True
                  )
                  rec = rpool.tile([128, 512], f32, tag="r",
                                   name=f"{_p}r_{h}_{qb}")
                  nc.vector.reciprocal(out=rec[:], in_=d_ps[:])
                  nc.vector.tensor_tensor(
                      at_a[:, h, qb * 512 : (qb + 1) * 512],
                      o_ps[:], rec[:], op=mul,
                  )

              def _out_mtile(m):
                  ot = outp.tile([128, NN, 512], f16, tag="ot", name=f"{_p}ot_{m}")
                  for n in range(NN):
                      xp = dxps.tile([128, 512], f32, tag="dx",
                                     name=f"{_p}x_{n}_{m}")
                      for k4 in range(_HPC):
                          nc.tensor.matmul(
                              xp[:],
                              at_a[:, k4, m * 128 : (m + 1) * 128],
                              wo_t[n][:, k4, :],
                              start=(k4 == 0), stop=(k4 == _HPC - 1),
                          )
                      if n % 2 == 0:
                          nc.scalar.copy(ot[:, n, :], xp[:])
                      else:
                          nc.vector.tensor_copy(ot[:, n, :], xp[:])
                  for d0 in range(2):
                      nc.sync.dma_start(
                          out_d[m * 128 : (m + 1) * 128,
                                d0 * 2048 : (d0 + 1) * 2048],
                          ot[:, d0 * 4 : (d0 + 1) * 4, :],
                      )

              # O chains trail attention by one query bank so PE can fill
              # ACT-exp bubbles; emit interleaved at head granularity.
              owork = []  # pending m-tiles
              for qb in range(NQB):
                  for h in range(_HPC):
                      if 'a' in phases:
                          _attn_head(h, qb)
                      if 'o' in phases and owork:
                          _out_mtile(owork.pop(0))
                  owork.extend(range(4 * qb, 4 * qb + 4))
              if 'o' in phases:
                  for m in owork:
                      _out_mtile(m)

    nc.compile()
    return nc


def _host_prep(hidden_states, position_ids, Wq, Wk, Wv, Wo, qn_w, qn_b, kn_w, kn_b):
    S = hidden_states.shape[1]
    hT = np.ascontiguousarray(
        np.asarray(hidden_states, np.float32)[0].T.astype(np.float16)
    )
    pos = np.asarray(position_ids, np.float32)[0]  # [S]
    inv = 1.0 / (_ROPE_THETA ** (np.arange(0, _D, 2, dtype=np.float32) / _D))
    fr = pos[:, None] * inv[None, :]  # [S, D/2]
    emb = np.concatenate([fr, fr], axis=1)  # [S, D]
    cos = np.cos(emb).astype(np.float32)
    sin = np.sin(emb).astype(np.float32)

    half = _D // 2
    perm = np.concatenate([np.arange(half, _D), np.arange(0, half)])
    sign = np.concatenate([-np.ones(half, np.float32), np.ones(half, np.float32)])

    def coeffs(g, b):
        g = np.asarray(g, np.float32).reshape(_D)
        b = np.asarray(b, np.float32).reshape(_D)
        c1 = g[None, :] * cos  # [S, D]
        dd = (sign * g[perm])[None, :] * sin
        ee = b[None, :] * cos + (sign * b[perm])[None, :] * sin
        return c1.astype(np.float16), dd.astype(np.float16), ee.astype(np.float16)

    c1q, ddq, eeq = coeffs(qn_w, qn_b)
    c1k, ddk, eek = coeffs(kn_w, kn_b)

    masks = np.zeros((4, 128, 512), np.float16)
    for t in range(4):
        kk = np.arange(128)[:, None] + t * 128
        qq = np.arange(512)[None, :]
        masks[t] = (kk <= qq).astype(np.float16)

    common = {
        "hT": hT,
        "c1q": c1q, "ddq": ddq, "eeq": eeq,
        "c1k": c1k, "ddk": ddk, "eek": eek,
        "masks": masks,
    }
    Wq = np.asarray(Wq, np.float32).astype(np.float16)
    Wk = np.asarray(Wk, np.float32).astype(np.float16)
    Wv = np.asarray(Wv, np.float32).astype(np.float16)
    Wo16 = np.asarray(Wo, np.float32).astype(np.float16)
    in_maps = []
    for c in range(_NC):
        sl = slice(c * _CPW, (c + 1) * _CPW)
        m = dict(common)
        m["wq"] = np.ascontiguousarray(Wq[:, sl])
        m["wk"] = np.ascontiguousarray(Wk[:, sl])
        m["wv"] = np.ascontiguousarray(Wv[:, sl])
        m["wo"] = np.ascontiguousarray(Wo16[sl, :])
        in_maps.append(m)
    return in_maps


def kernel(**inputs) -> np.ndarray:
    from concourse.bass_utils import run_bass_kernel_spmd

    hidden_states = np.asarray(inputs["hidden_states"])
    S = hidden_states.shape[1]
    if S not in _cache:
        _cache[S] = _build(S)
    nc = _cache[S]

    in_maps = _host_prep(
        hidden_states,
        inputs["position_ids"],
        inputs["Wq"], inputs["Wk"], inputs["Wv"], inputs["Wo"],
        inputs["qn_w"], inputs["qn_b"], inputs["kn_w"], inputs["kn_b"],
    )
    res = run_bass_kernel_spmd(nc, in_maps, list(range(_NC)))
    out = np.zeros((S, _HID), np.float32)
    for c in range(_NC):
        out += res.results[c]["out"].astype(np.float32)
    return out.reshape(1, S, _HID)


# revision 14
# speedup vs baseline: 2.3350x; 2.3350x over previous
"""ChameleonAttention on 8 Trainium2 NeuronCores.

Tensor-parallel over heads: each core owns 4 of the 32 heads.
  - Wq/Wk/Wv sharded column-wise (512 cols/core), Wo row-wise (512 rows/core)
  - per-head LayerNorm + RoPE computed on-chip, gamma/beta replicated
  - causal attention with block-skipping (only lower-triangular key tiles)
  - per-core partial output [S, HID] summed on host (the TP all-reduce)

v3 engine balance (cost-model driven):
  - PE ~519us is the floor; every other engine is kept under it
  - projection operands fp16 (same PE rate as f32r, half DMA/SBUF)
  - denominator via DVE/fp16 u-sum + one ones-matmul per (head, bank)
  - exp batched over [128,1024] two-bank PSUM score groups
  - O-projection chains interleaved between attention head-blocks,
    offset by one query bank, so PE fills ACT-exp bubbles
  - PSUM->SBUF copies on ACT (scalar.copy), RoPE t24 on gpsimd, rest of
    RoPE + LN on DVE: DVE ~280us, ACT ~250us, Pool ~65us
  - DMAs merged (w per k-panel, rope per half, wo per n-block, out per
    m-tile) to cut SP dispatch serialization from ~290us to ~70us

Softmax uses exp(s*scale - 4) with no running max (LayerNormed q/k bound
the logits), division deferred to after the P@V accumulation.

RoPE is folded with the LayerNorm affine on the host:
  q'[s,j] = xn[s,j]*C1[s,j] + xn[s,p(j)]*D[s,j] + E[s,j]
with C1 = gamma*cos, D = sign*gamma[perm]*sin, E = beta*cos +
sign*beta[perm]*sin.
"""
import math
from contextlib import ExitStack

import numpy as np

_S = 2048
_HID = 4096
_D = 128
_NC = 8
_CPW = _HID // _NC  # columns per core (512) = 4 heads
_HPC = _CPW // _D  # heads per core (4)
_KP = 1024  # contraction panel (8 k-tiles of 128)
_NPAN = _HID // _KP  # 4 panels
_ROPE_THETA = 10000.0
_EPS = 1e-5
_EXP_BIAS = -4.0

_cache = {}


def _build(S, niter=1, phases=('p', 'a', 'o'), knobs=None):
    kb = {'proj': 14, 'wpool': 7, 'pps': 2, 'tps': 2, 'upool': 5, 'sps': 2,
          'ops': 2, 'dxps': 2, 'lnbig': 3, 'lntmp': 4, 'ropep': 1,
          'wop': 8, 'outp': 3}
    kb.update(knobs or {})
    import concourse.tile as tile
    from concourse import bacc, mybir
    from concourse.masks import make_identity

    f32 = mybir.dt.float32
    f16 = mybir.dt.float16
    mul = mybir.AluOpType.mult
    add = mybir.AluOpType.add

    NM = S // 128  # s-tiles (16)
    NQB = S // 512  # query banks (4)
    NMH = NM // 2  # s-tiles per half (8)
    SH = S // 2  # rows per half
    KPT = _KP // 128  # k-tiles per panel (8)
    NN = _HID // 512  # output column blocks (8)

    nc = bacc.Bacc("TRN2", target_bir_lowering=False, debug=False)

    hT_d = nc.dram_tensor("hT", [_HID, S], f16, kind="ExternalInput")
    w_d = {
        t: nc.dram_tensor(f"w{t}", [_HID, _CPW], f16, kind="ExternalInput")
        for t in ("q", "k", "v")
    }
    wo_d = nc.dram_tensor("wo", [_CPW, _HID], f16, kind="ExternalInput")
    rope_d = {}
    for t in ("q", "k"):
        for nm in ("c1", "dd", "ee"):
            rope_d[nm + t] = nc.dram_tensor(
                f"{nm}{t}", [S, _D], f16, kind="ExternalInput"
            )
    masks_d = nc.dram_tensor("masks", [4, 128, 512], f16, kind="ExternalInput")
    out_d = nc.dram_tensor("out", [S, _HID], f16, kind="ExternalOutput")

    half_d = _D // 2

    for _it in range(niter):
      _p = f'i{_it}_' if niter > 1 else ''
      with tile.TileContext(nc) as tc, ExitStack() as ctx:
          # ---- persistent small constants ----
          persist = ctx.enter_context(tc.tile_pool(name=f"{_p}persist", bufs=1))
          ident16 = persist.tile([128, 128], f16)
          make_identity(nc, ident16[:])
          ones16 = persist.tile([128, 128], f16)
          nc.vector.memset(ones16[:], 1.0)
          ebias = persist.tile([128, 1], f32)
          nc.vector.memset(ebias[:], _EXP_BIAS)
          epst = persist.tile([128, 1], f32)
          nc.vector.memset(epst[:], _EPS)

          # ---- fp16 attention operands, filled by phase P ----
          att = ctx.enter_context(tc.tile_pool(name=f"{_p}att", bufs=1))
          qt_a = att.tile([128, _HPC, S], f16, name=f"{_p}qt")
          kt_a = att.tile([128, _HPC, S], f16, name=f"{_p}kt")
          v16 = att.tile([128, NM, 512], f16)

          wop = ctx.enter_context(tc.tile_pool(name=f"{_p}wop", bufs=kb["wop"]))
          wo_t = []  # wo_t[n] = [128, 4, 512]: Wo[k4*128+p, n*512+j]
          mpool = ctx.enter_context(tc.tile_pool(name=f"{_p}mpool", bufs=1))
          mask_t = mpool.tile([128, 4, 512], f16)
          nc.sync.dma_start(mask_t[:], masks_d.ap().rearrange("t p n -> p t n"))

          # ================= phase P: QKV projection =================
          with ExitStack() as pctx:
              acc_pool = pctx.enter_context(tc.tile_pool(name=f"{_p}acc", bufs=1))
              acc = {}
              for t in ("q", "k", "v"):
                  for m in range(NMH):
                      acc[(t, m)] = acc_pool.tile(
                          [128, 512], f16, name=f"{_p}acc_{t}{m}"
                      )
              proj = pctx.enter_context(tc.tile_pool(name=f"{_p}proj", bufs=kb["proj"]))
              wpool = pctx.enter_context(tc.tile_pool(name=f"{_p}wpool", bufs=kb["wpool"]))
              lnbig = pctx.enter_context(tc.tile_pool(name=f"{_p}lnbig", bufs=kb["lnbig"]))
              lntmp = pctx.enter_context(tc.tile_pool(name=f"{_p}lntmp", bufs=kb["lntmp"]))
              ropep = pctx.enter_context(tc.tile_pool(name=f"{_p}ropep", bufs=kb["ropep"]))
              pps = pctx.enter_context(tc.tile_pool(name=f"{_p}pps", bufs=kb["pps"], space="PSUM"))
              tps = pctx.enter_context(tc.tile_pool(name=f"{_p}tps", bufs=kb["tps"], space="PSUM"))

              # rope coeff tiles, one merged DMA per (coeff, tensor, half):
              # [128, 8, 128] with free dim (m, d)
              rope_t = {}

              def _load_rope(half):
                  for t in ("q", "k"):
                      for nm in ("c1", "dd", "ee"):
                          tl = ropep.tile([128, NMH, _D], f16, tag=f"{nm}{t}",
                                          name=f"{_p}rope_{nm}{t}{half}")
                          nc.sync.dma_start(
                              tl[:],
                              rope_d[nm + t][half * SH : (half + 1) * SH, :]
                              .rearrange("(m p) d -> p m d", p=128),
                          )
                          rope_t[(nm, t, half)] = tl

              def _ln_rope(t, half, m):
                  dst = qt_a[:] if t == "q" else kt_a[:]
                  gm = half * NMH + m
                  c1 = rope_t[("c1", t, half)][:, m, :]
                  dd = rope_t[("dd", t, half)][:, m, :]
                  ee = rope_t[("ee", t, half)][:, m, :]
                  xn4 = lnbig.tile([128, _HPC, _D], f16, tag="xn4",
                                   name=f"{_p}xn4_{half}{t}{m}")
                  for h in range(_HPC):
                      x = acc[(t, m)][:, h * _D : (h + 1) * _D]
                      st = lntmp.tile([128, 6], f32, tag="st",
                                      name=f"{_p}st_{half}{t}{m}{h}")
                      mv = lntmp.tile([128, 2], f32, tag="mv",
                                      name=f"{_p}mv_{half}{t}{m}{h}")
                      nc.vector.bn_stats(out=st[:], in_=x)
                      nc.vector.bn_aggr(out=mv[:], in_=st[:])
                      rstd = lntmp.tile([128, 1], f32, tag="rs",
                                        name=f"{_p}rs_{half}{t}{m}{h}")
                      nc.scalar.activation(
                          out=rstd[:], in_=mv[:, 1:2],
                          func=mybir.ActivationFunctionType.Sqrt,
                          bias=epst[:], scale=1.0,
                      )
                      nc.vector.reciprocal(out=rstd[:], in_=rstd[:])
                      nc.vector.tensor_scalar(
                          out=xn4[:, h, :], in0=x,
                          scalar1=mv[:, 0:1], scalar2=rstd[:],
                          op0=mybir.AluOpType.subtract, op1=mul,
                      )
                  # rope: q' = xn*C1 + rot(xn)*D + E, rot via shifted slices.
                  # t24 (gpsimd): rot(xn)*D + E ; q14 (DVE): xn*C1 + t24
                  t24 = lnbig.tile([128, _HPC, _D], f16, tag="t24",
                                   name=f"{_p}t24_{half}{t}{m}")
                  q14 = lnbig.tile([128, _HPC, _D], f16, tag="q14",
                                   name=f"{_p}q14_{half}{t}{m}")
                  c1b = c1.unsqueeze(1).broadcast_to((128, _HPC, _D))
                  ddlo = dd[:, :half_d].unsqueeze(1) \
                      .broadcast_to((128, _HPC, half_d))
                  ddhi = dd[:, half_d:].unsqueeze(1) \
                      .broadcast_to((128, _HPC, half_d))
                  eeb = ee.unsqueeze(1).broadcast_to((128, _HPC, _D))
                  nc.gpsimd.tensor_tensor(
                      t24[:, :, :half_d], xn4[:, :, half_d:], ddlo, op=mul
                  )
                  nc.gpsimd.tensor_tensor(
                      t24[:, :, half_d:], xn4[:, :, :half_d], ddhi, op=mul
                  )
                  nc.gpsimd.tensor_tensor(t24[:], t24[:], eeb, op=add)
                  nc.vector.tensor_tensor(q14[:], xn4[:], c1b, op=mul)
                  nc.vector.tensor_tensor(q14[:], q14[:], t24[:], op=add)
                  tp = tps.tile([128, _HPC, _D], f16, tag="tp",
                                name=f"{_p}tp_{half}{t}{m}")
                  for h in range(_HPC):
                      nc.tensor.transpose(tp[:, h, :], q14[:, h, :], ident16[:])
                  nc.scalar.copy(dst[:, :, gm * 128 : (gm + 1) * 128], tp[:])

              early_w = {}
              for half in range(2):
                  for kp in range(_NPAN):
                      if kp == 1:
                          _load_rope(half)
                      if half == 0 and kp == 0:
                          for wh in range(2):
                              wt = wpool.tile([128, KPT // 2, 512], f16, tag="w",
                                              name=f"{_p}wq_early_{wh}")
                              nc.sync.dma_start(
                                  wt[:],
                                  w_d["q"][wh * (_KP // 2) : (wh + 1) * (_KP // 2), :]
                                  .rearrange("(k p) n -> p k n", p=128),
                              )
                              early_w[wh] = wt
                      hts = []
                      for k4 in range(KPT):
                          ht = proj.tile([128, SH], f16, tag="ht",
                                         name=f"{_p}ht_{half}_{kp}_{k4}")
                          kk = kp * KPT + k4
                          nc.sync.dma_start(
                              ht[:],
                              hT_d[kk * 128 : (kk + 1) * 128,
                                   half * SH : (half + 1) * SH],
                          )
                          hts.append(ht)
                      if half == 0 and kp == 1:
                          # Wo loads: early enough to be resident by phase
                          # A/O, late enough not to delay the first panel
                          for n in range(NN):
                              t_ = wop.tile([128, _HPC, 512], f16, tag="wo",
                                            name=f"{_p}wo_{n}")
                              nc.sync.dma_start(
                                  t_[:],
                                  wo_d[:, n * 512 : (n + 1) * 512]
                                  .rearrange("(k p) n -> p k n", p=128),
                              )
                              wo_t.append(t_)
                      wts_t = {}
                      for t in ("q", "k", "v"):
                          # merged weight panels: 2 x [128, 4, 512] = (k4, n)
                          if half == 0 and kp == 0 and t == "q":
                              wts_t[t] = [early_w[0], early_w[1]]
                          else:
                              wts = []
                              for wh in range(2):
                                  wt = wpool.tile([128, KPT // 2, 512], f16, tag="w",
                                                  name=f"{_p}w{t}_{half}_{kp}_{wh}")
                                  r0 = kp * _KP + wh * (_KP // 2)
                                  nc.sync.dma_start(
                                      wt[:],
                                      w_d[t][r0 : r0 + _KP // 2, :]
                                      .rearrange("(k p) n -> p k n", p=128),
                                  )
                                  wts.append(wt)
                              wts_t[t] = wts
                      last = kp == _NPAN - 1
                      # on the last panel, round-robin (m, t) so the serial
                      # LN->rope->transpose tails overlap remaining chains
                      if last:
                          order = [(t, m) for m in range(NMH)
                                   for t in ("q", "k", "v")]
                      else:
                          order = [(t, m) for t in ("q", "k", "v")
                                   for m in range(NMH)]
                      for t, m in order:
                          wts = wts_t[t]
                          ps = pps.tile([128, 512], f32, tag="ps",
                                        name=f"{_p}ps_{half}_{kp}_{t}_{m}")
                          for k4 in range(KPT):
                              nc.tensor.matmul(
                                  ps[:],
                                  hts[k4][:, m * 128 : (m + 1) * 128],
                                  wts[k4 // 4][:, k4 % 4, :],
                                  start=(k4 == 0),
                                  stop=(k4 == KPT - 1),
                              )
                          if kp == 0:
                              nc.scalar.copy(acc[(t, m)][:], ps[:])
                          elif last and t == "v":
                              # final V panel: fuse add + fp16 cast
                              gm = half * NMH + m
                              nc.vector.tensor_tensor(
                                  v16[:, gm, :], acc[(t, m)][:], ps[:], op=add
                              )
                          else:
                              nc.vector.tensor_tensor(
                                  acc[(t, m)][:], acc[(t, m)][:], ps[:], op=add
                              )
                          if last and t != "v":
                              _ln_rope(t, half, m)

          # ============ phases A+O, O interleaved one bank behind ============
          if 'a' not in phases and 'o' not in phases:
              continue
          with ExitStack() as actx:
              aop = actx.enter_context(tc.tile_pool(name=f"{_p}aop", bufs=1))
              at_a = aop.tile([128, _HPC, S], f16, name=f"{_p}at")
              if 'a' not in phases:
                  nc.vector.memset(at_a[:], 0.0)

              upool = actx.enter_context(tc.tile_pool(name=f"{_p}upool", bufs=kb["upool"]))
              rpool = actx.enter_context(tc.tile_pool(name=f"{_p}rpool", bufs=2))
              outp = actx.enter_context(tc.tile_pool(name=f"{_p}outp", bufs=kb["outp"]))
              sps = actx.enter_context(tc.tile_pool(name=f"{_p}sps", bufs=kb["sps"], space="PSUM"))
              ops = actx.enter_context(tc.tile_pool(name=f"{_p}ops", bufs=kb["ops"], space="PSUM"))
              dxps = actx.enter_context(tc.tile_pool(name=f"{_p}dxps", bufs=kb["dxps"], space="PSUM"))

              scale = 1.0 / math.sqrt(_D)

              def _attn_head(h, qb):
                  nkt = 4 * qb + 4
                  npair = nkt // 2
                  o_ps = ops.tile([128, 512], f32, tag="o", name=f"{_p}o_{h}_{qb}")
                  usum = upool.tile([128, 512], f16, tag="usum",
                                    name=f"{_p}us_{h}_{qb}")
                  for pr in range(npair):
                      sg = sps.tile([128, 1024], f32, tag="s",
                                    name=f"{_p}s_{h}_{qb}_{pr}")
                      for j in range(2):
                          kt = 2 * pr + j
                          nc.tensor.matmul(
                              sg[:, j * 512 : (j + 1) * 512],
                              kt_a[:, h, kt * 128 : (kt + 1) * 128],
                              qt_a[:, h, qb * 512 : (qb + 1) * 512],
                              start=True, stop=True,
                          )
                      u = upool.tile([128, 1024], f16, tag="u",
                                     name=f"{_p}u_{h}_{qb}_{pr}")
                      nc.scalar.activation(
                          out=u[:], in_=sg[:],
                          func=mybir.ActivationFunctionType.Exp,
                          bias=ebias[:], scale=scale,
                      )
                      for j in range(2):
                          kt = 2 * pr + j
                          toff = kt - 4 * qb
                          if toff >= 0:
                              nc.vector.tensor_tensor(
                                  u[:, j * 512 : (j + 1) * 512],
                                  u[:, j * 512 : (j + 1) * 512],
                                  mask_t[:, toff, :], op=mul,
                              )
                      if pr == 0:
                          nc.vector.tensor_tensor(
                              usum[:], u[:, :512], u[:, 512:], op=add
                          )
                      else:
                          for j in range(2):
                              nc.vector.tensor_tensor(
                                  usum[:], usum[:],
                                  u[:, j * 512 : (j + 1) * 512], op=add,
                              )
                      for j in range(2):
                          kt = 2 * pr + j
                          nc.tensor.matmul(
                              o_ps[:],
                              v16[:, kt, h * _D : (h + 1) * _D],
                              u[:, j * 512 : (j + 1) * 512],
                              start=(kt == 0), stop=(kt == nkt - 1),
                          )
                  d_ps = ops.tile([128, 512], f32, tag="o",
                                  name=f"{_p}d_{h}_{qb}")
                  nc.tensor.matmul(
                      d_ps[:], ones16[:], usum[:], start=True, stop=True
                  )
                  rec = rpool.tile([128, 512], f32, tag="r",
                                   name=f"{_p}r_{h}_{qb}")
                  nc.vector.reciprocal(out=rec[:], in_=d_ps[:])
                  nc.vector.tensor_tensor(
                      at_a[:, h, qb * 512 : (qb + 1) * 512],
                      o_ps[:], rec[:], op=mul,
                  )

              def _out_mtile(m):
                  ot = outp.tile([128, NN, 512], f16, tag="ot", name=f"{_p}ot_{m}")
                  for n in range(NN):
                      xp = dxps.tile([128, 512], f32, tag="dx",
                                     name=f"{_p}x_{n}_{m}")
                      for k4 in range(_HPC):
                          nc.tensor.matmul(
                              xp[:],
                              at_a[:, k4, m * 128 : (m + 1) * 128],
                              wo_t[n][:, k4, :],
                              start=(k4 == 0), stop=(k4 == _HPC - 1),
                          )
                      if n % 2 == 0:
                          nc.scalar.copy(ot[:, n, :], xp[:])
                      else:
                          nc.vector.tensor_copy(ot[:, n, :], xp[:])
                  nsplit = 8 if m == NM - 1 else 2
                  step = NN // nsplit
                  for d0 in range(nsplit):
                      nc.sync.dma_start(
                          out_d[m * 128 : (m + 1) * 128,
                                d0 * step * 512 : (d0 + 1) * step * 512],
                          ot[:, d0 * step : (d0 + 1) * step, :],
                      )

              # O chains trail attention by one query bank so PE can fill
              # ACT-exp bubbles; emit interleaved at head granularity.
              owork = []  # pending m-tiles
              for qb in range(NQB):
                  for h in range(_HPC):
                      if 'a' in phases:
                          _attn_head(h, qb)
                      if 'o' in phases and owork:
                          _out_mtile(owork.pop(0))
                  owork.extend(range(4 * qb, 4 * qb + 4))
              if 'o' in phases:
                  for m in owork:
                      _out_mtile(m)

    nc.compile()
    return nc


def _host_prep(hidden_states, position_ids, Wq, Wk, Wv, Wo, qn_w, qn_b, kn_w, kn_b):
    S = hidden_states.shape[1]
    hT = np.ascontiguousarray(
        np.asarray(hidden_states, np.float32)[0].T.astype(np.float16)
    )
    pos = np.asarray(position_ids, np.float32)[0]  # [S]
    inv = 1.0 / (_ROPE_THETA ** (np.arange(0, _D, 2, dtype=np.float32) / _D))
    fr = pos[:, None] * inv[None, :]  # [S, D/2]
    emb = np.concatenate([fr, fr], axis=1)  # [S, D]
    cos = np.cos(emb).astype(np.float32)
    sin = np.sin(emb).astype(np.float32)

    half = _D // 2
    perm = np.concatenate([np.arange(half, _D), np.arange(0, half)])
    sign = np.concatenate([-np.ones(half, np.float32), np.ones(half, np.float32)])

    def coeffs(g, b):
        g = np.asarray(g, np.float32).reshape(_D)
        b = np.asarray(b, np.float32).reshape(_D)
        c1 = g[None, :] * cos  # [S, D]
        dd = (sign * g[perm])[None, :] * sin
        ee = b[None, :] * cos + (sign * b[perm])[None, :] * sin
        return c1.astype(np.float16), dd.astype(np.float16), ee.astype(np.float16)

    c1q, ddq, eeq = coeffs(qn_w, qn_b)
    c1k, ddk, eek = coeffs(kn_w, kn_b)

    masks = np.zeros((4, 128, 512), np.float16)
    for t in range(4):
        kk = np.arange(128)[:, None] + t * 128
        qq = np.arange(512)[None, :]
        masks[t] = (kk <= qq).astype(np.float16)

    common = {
        "hT": hT,
        "c1q": c1q, "ddq": ddq, "eeq": eeq,
        "c1k": c1k, "ddk": ddk, "eek": eek,
        "masks": masks,
    }
    Wq = np.asarray(Wq, np.float32).astype(np.float16)
    Wk = np.asarray(Wk, np.float32).astype(np.float16)
    Wv = np.asarray(Wv, np.float32).astype(np.float16)
    Wo16 = np.asarray(Wo, np.float32).astype(np.float16)
    in_maps = []
    for c in range(_NC):
        sl = slice(c * _CPW, (c + 1) * _CPW)
        m = dict(common)
        m["wq"] = np.ascontiguousarray(Wq[:, sl])
        m["wk"] = np.ascontiguousarray(Wk[:, sl])
        m["wv"] = np.ascontiguousarray(Wv[:, sl])
        m["wo"] = np.ascontiguousarray(Wo16[sl, :])
        in_maps.append(m)
    return in_maps


def kernel(**inputs) -> np.ndarray:
    from concourse.bass_utils import run_bass_kernel_spmd

    hidden_states = np.asarray(inputs["hidden_states"])
    S = hidden_states.shape[1]
    if S not in _cache:
        _cache[S] = _build(S)
    nc = _cache[S]

    in_maps = _host_prep(
        hidden_states,
        inputs["position_ids"],
        inputs["Wq"], inputs["Wk"], inputs["Wv"], inputs["Wo"],
        inputs["qn_w"], inputs["qn_b"], inputs["kn_w"], inputs["kn_b"],
    )
    res = run_bass_kernel_spmd(nc, in_maps, list(range(_NC)))
    out = np.zeros((S, _HID), np.float32)
    for c in range(_NC):
        out += res.results[c]["out"].astype(np.float32)
    return out.reshape(1, S, _HID)
